# revision 48
# baseline (speedup 1.0000x reference)
"""Distributed KNN online evaluator kernel for 8 trn2 NeuronCores.

Device side (SPMD over 8 cores, bank sharded over N):
  - bank shard (+queries) resident in SBUF as fp8e4m3, loaded via 7
    chained DMAs (fp8 halves HBM traffic; margin covers the quantization)
  - fp8 matmuls (queries stationary per chunk) -> f32 PSUM,
    1024-col groups on a ring-4 PSUM (fine granularity keeps the
    tensor/ACT/DVE pipeline decoupled; coarser evac units serialize)
  - evacuation split 16 ACT-groups : 8 DVE-groups (+tail) per chunk:
      * D-groups: DVE tensor_reduce blockmax-4 straight from PSUM
      * A-groups: ACT f32->bf16 copy to an 8-slot SBUF stage ring; DVE
        folds 4 staged groups per quad with a 2-level TT max tree
        (2x packed bf16) -> blockmax-4
  - batched DMA out of per-(query, block-of-4) maxima as bf16

Host side:
  - adaptive drill-down: select blocks whose blockmax could contain a
    global top-K sim, recompute those sims exactly in f32, take top-K
  - verified: every unselected block provably below the top-K threshold
    (margin covers fp8/matmul fuzz); expands selection until proven
  - class votes with inf weights degenerate to membership -> output is
    [voted classes asc, unvoted classes asc] per query
"""

import numpy as np
import ml_dtypes

import concourse.bass as bass
import concourse.mybir as mybir
from concourse.bass_utils import run_bass_kernel_spmd

BF16 = ml_dtypes.bfloat16
FP8 = ml_dtypes.float8_e4m3fn

N_CORES = 8
B = 256
D = 128
N_TOTAL = 200000
N_SHARD = N_TOTAL // N_CORES   # 25000
GROUP = 1024
N_FULL = 24
TAIL = 512
NCOL = N_FULL * GROUP + TAIL   # 25088
QOFF = 256
N_STEPS_C = N_FULL + 1
BLK = 4
SLOTS_C = NCOL // BLK          # 6272 total screen slots per chunk
N_RAW = 13                     # A-groups per chunk shipped raw to HBM
N_FOLD = 0                     # no DVE folds: every A-group ships raw
SLOTS_DEV = SLOTS_C - N_RAW * (GROUP // BLK)   # 2944 device blockmax slots
RAW_RING = 8                   # rawbuf ring slots x 1024 bf16
K = 200
NUM_CLASSES = 1000
MARGIN = 3.5                   # fp8 sim err measured |max| 2.7 over 12.8M

PSUM_RING = 4
STAGE_RING = 8                 # stage: 8 slots x 1024 bf16
# 14 A + 10 D per chunk (+tail): ACT is the binding engine at 16 A
# (proven: removing DVE work alone changed nothing), so shift 2 groups
# to DVE direct-reduce and keep DVE in budget by shipping 6 of the 14
# A-groups raw to HBM (host blockmaxes them) instead of folding.
_DSET = {0, 2, 4, 7, 9, 11, 13, 15, 17, 20, 22}
ASSIGN = ['D' if i in _DSET else 'A' for i in range(24)]
# bank DMA parts (cols, queue): alternate the HWDGE (sync) and SWDGE
# (gpsimd) rings so one ring's ~2us completion receipt overlaps the
# other ring's transfer (single-ring parts arrived only every ~3.5us).
DMA_PARTS = [(QOFF + GROUP, 's'), (3 * GROUP, 'g'), (3 * GROUP, 's'),
             (3 * GROUP, 'g'), (3 * GROUP, 's'), (3 * GROUP, 'g'),
             (3 * GROUP, 's'), (3 * GROUP, 'g'), (2 * GROUP + TAIL, 's')]
assert sum(c for c, _ in DMA_PARTS) == QOFF + NCOL
OUT_BATCH = 3                  # DVE output units per out-DMA

_NC_CACHE = None


def _plan():
    steps = []
    for c in range(2):
        na_c = 0
        for g in range(N_STEPS_C):
            kind = 'T' if g == N_FULL else ASSIGN[g]
            st = dict(c=c, g=g, kind=kind, s=len(steps))
            if kind == 'A':
                na_c += 1
                # first N_FOLD A-groups fold on DVE, the rest ship raw
                st['raw'] = na_c > N_FOLD
            steps.append(st)
    na = nf = nr = 0
    for st in steps:
        if st['kind'] == 'A':
            na += 1
            st['na'] = na          # global A order (evacA threshold)
            if st['raw']:
                st['nr'] = nr      # raw copy order (rawbuf slot / DMA)
                nr += 1
            else:
                st['nf'] = nf      # fold copy order (stage slot)
                nf += 1
    # DVE atoms: directs get priority (they recycle PSUM ring slots);
    # quad folds are split into f1/f2 atoms drained one per direct so no
    # single DVE block delays the ring by more than ~1.2us.
    dve_ops = []
    for c in range(2):
        pend = []
        fold_fifo = []
        for st in steps:
            if st['c'] != c:
                continue
            if st['kind'] in ('D', 'T'):
                dve_ops.append(dict(op='direct', st=st))
                if fold_fifo:
                    dve_ops.append(fold_fifo.pop(0))
            elif not st['raw']:
                pend.append(st)
                if len(pend) == 4:
                    q = dict(grp=pend)
                    fold_fifo.append(dict(op='f1', q=q))
                    fold_fifo.append(dict(op='f2', q=q))
                    pend = []
        dve_ops.extend(fold_fifo)
        assert not pend
    outs = []
    base = {0: 0, 1: 0}
    nd = nu = 0
    for op in dve_ops:
        if op['op'] == 'direct':
            st = op['st']
            c = st['c']
            nslots = (TAIL if st['kind'] == 'T' else GROUP) // BLK
            nd += 1
            outs.append(dict(op=op, c=c, base=base[c], nslots=nslots,
                             sem='evacD', cnt=nd))
        elif op['op'] == 'f2':
            c = op['q']['grp'][0]['c']
            nslots = 1024
            nu += 1
            outs.append(dict(op=op, c=c, base=base[c], nslots=nslots,
                             sem='out', cnt=nu))
        else:
            continue
        op['out'] = outs[-1]
        base[c] += outs[-1]['nslots']
    assert base[0] == SLOTS_DEV and base[1] == SLOTS_DEV, base
    # batch consecutive outs (same chunk) into single DMAs
    batches = []
    cur = []
    for o in outs:
        if cur and (len(cur) >= OUT_BATCH or cur[-1]['c'] != o['c']):
            batches.append(cur)
            cur = []
        cur.append(o)
    if cur:
        batches.append(cur)
    return steps, dve_ops, outs, batches


def _part_of(g):
    acc = -QOFF
    for pi, (cols, _) in enumerate(DMA_PARTS):
        acc += cols
        if g * GROUP < acc:
            return pi
    return len(DMA_PARTS) - 1


def _dma_need(p):
    """(n_sync_parts, n_gp_parts) among parts 0..p inclusive."""
    ns = sum(1 for c, q in DMA_PARTS[:p + 1] if q == 's')
    ng = (p + 1) - ns
    return ns, ng


def _build_nc():
    steps, dve_ops, outs, batches = _plan()
    nc = bass.Bass("TRN2", target_bir_lowering=False, debug=False,
                   num_devices=N_CORES)
    bankT = nc.dram_tensor("bankT", [D, QOFF + NCOL], mybir.dt.float8e4,
                           kind="ExternalInput").ap()
    out = nc.dram_tensor("blockmax", [B, SLOTS_DEV], mybir.dt.bfloat16,
                         kind="ExternalOutput").ap()
    raws = nc.dram_tensor("rawsims", [B, N_RAW * GROUP], mybir.dt.bfloat16,
                          kind="ExternalOutput").ap()

    MAX = mybir.AluOpType.max

    with (
        nc.sbuf_tensor([D, QOFF + NCOL], mybir.dt.float8e4) as banks,
        nc.psum_tensor([128, PSUM_RING * GROUP], mybir.dt.float32) as psum,
        nc.sbuf_tensor([128, STAGE_RING * GROUP], mybir.dt.bfloat16) as stage,
        nc.sbuf_tensor([128, 2048], mybir.dt.bfloat16) as m1,
        nc.sbuf_tensor([128, 2 * SLOTS_DEV], mybir.dt.bfloat16) as obuf,
        nc.sbuf_tensor([128, RAW_RING * GROUP], mybir.dt.bfloat16) as rawbuf,
        nc.semaphore() as raw_free,
        nc.semaphore() as dmaS_sem,
        nc.semaphore() as dmaG_sem,
        nc.semaphore() as dmao_sem,
        nc.semaphore() as mm_sem,
        nc.semaphore() as evacA,
        nc.semaphore() as evacD,
        nc.semaphore() as out_sem,
        nc.semaphore() as stage_free,
        nc.Block() as block,
    ):
        @block.gpsimd
        def _(gp):
            lo = 0
            for cols, q in DMA_PARTS:
                if q == 'g':
                    gp.dma_start(banks[:, lo:lo + cols],
                                 bankT[:, lo:lo + cols]).then_inc(dmaG_sem, 16)
                lo += cols
            # raw sims: ship ACT-staged bf16 groups to HBM; host blockmaxes
            for st in steps:
                if st['kind'] != 'A' or not st['raw']:
                    continue
                i, c = st['nr'], st['c']
                gp.wait_ge(evacA, st['na'])
                gp.dma_start(
                    raws[c * 128:(c + 1) * 128,
                         (i % N_RAW) * GROUP:(i % N_RAW) * GROUP + GROUP],
                    rawbuf[:, (i % RAW_RING) * GROUP:
                           (i % RAW_RING) * GROUP + GROUP]
                    ).then_inc(raw_free, 16)

        @block.sync
        def _(sync):
            lo = 0
            for cols, q in DMA_PARTS:
                if q == 's':
                    sync.dma_start(banks[:, lo:lo + cols],
                                   bankT[:, lo:lo + cols]
                                   ).then_inc(dmaS_sem, 16)
                lo += cols
            for batch in batches:
                need_d = max([o['cnt'] for o in batch
                              if o['sem'] == 'evacD'], default=0)
                need_u = max([o['cnt'] for o in batch
                              if o['sem'] == 'out'], default=0)
                if need_d:
                    sync.wait_ge(evacD, need_d)
                if need_u:
                    sync.wait_ge(out_sem, need_u)
                c = batch[0]['c']
                b0 = batch[0]['base']
                ns = sum(o['nslots'] for o in batch)
                sync.dma_start(out[c * 128:(c + 1) * 128, b0:b0 + ns],
                               obuf[:, c * SLOTS_DEV + b0:c * SLOTS_DEV + b0 + ns]
                               ).then_inc(dmao_sem, 16)

        @block.tensor
        def _(tensor):
            nA = nD = 0
            evac_of = {}
            wS = wG = 0
            for st in steps:
                s, c, g, kind = st['s'], st['c'], st['g'], st['kind']
                ns_, ng_ = _dma_need(_part_of(g))
                if ns_ > wS:
                    tensor.wait_ge(dmaS_sem, 16 * ns_)
                    wS = ns_
                if ng_ > wG:
                    tensor.wait_ge(dmaG_sem, 16 * ng_)
                    wG = ng_
                if s >= PSUM_RING:
                    sk, cv = evac_of[s - PSUM_RING]
                    tensor.wait_ge(evacA if sk == 'A' else evacD, cv)
                if kind == 'A':
                    nA += 1
                    evac_of[s] = ('A', nA)
                else:
                    nD += 1
                    evac_of[s] = ('D', nD)
                sl = (s % PSUM_RING) * GROUP
                cols = TAIL if kind == 'T' else GROUP
                nmm = cols // 512
                for m in range(nmm):
                    mm = tensor.matmul(
                        psum[:, sl + m * 512: sl + (m + 1) * 512],
                        lhsT=banks[:, c * 128:(c + 1) * 128],
                        rhs=banks[:, QOFF + g * GROUP + m * 512:
                                  QOFF + g * GROUP + (m + 1) * 512],
                        start=True, stop=True)
                    if m == nmm - 1:
                        mm.then_inc(mm_sem, 1)

        @block.scalar
        def _(scalar):
            for st in steps:
                if st['kind'] != 'A':
                    continue
                s = st['s']
                sl = (s % PSUM_RING) * GROUP
                if st['raw']:
                    i = st['nr']
                    if i >= RAW_RING:
                        scalar.wait_ge(raw_free, 16 * (i - (RAW_RING - 1)))
                    scalar.wait_ge(mm_sem, s + 1)
                    rs = (i % RAW_RING) * GROUP
                    scalar.copy(rawbuf[:, rs:rs + GROUP],
                                psum[:, sl:sl + GROUP]).then_inc(evacA, 1)
                else:
                    n = st['nf']
                    if n >= STAGE_RING:
                        scalar.wait_ge(stage_free, n - (STAGE_RING - 1))
                    scalar.wait_ge(mm_sem, s + 1)
                    ss = (n % STAGE_RING) * GROUP
                    scalar.copy(stage[:, ss:ss + GROUP],
                                psum[:, sl:sl + GROUP]).then_inc(evacA, 1)

        @block.vector
        def _(vector):
            for op in dve_ops:
                if op['op'] == 'direct':
                    o = op['out']
                    ob = obuf[:, o['c'] * SLOTS_DEV + o['base']:
                              o['c'] * SLOTS_DEV + o['base'] + o['nslots']]
                    st = op['st']
                    vector.wait_ge(mm_sem, st['s'] + 1)
                    sl = (st['s'] % PSUM_RING) * GROUP
                    cols = TAIL if st['kind'] == 'T' else GROUP
                    vector.tensor_reduce(
                        out=ob,
                        in_=psum[:, sl:sl + cols].rearrange(
                            "p (b w) -> p b w", w=BLK),
                        axis=mybir.AxisListType.X,
                        op=MAX,
                    ).then_inc(evacD, 1)
                elif op['op'] == 'f1':
                    grp = op['q']['grp']
                    vector.wait_ge(evacA, grp[-1]['na'])
                    ra = (grp[0]['nf'] % STAGE_RING) * GROUP
                    rb = (grp[2]['nf'] % STAGE_RING) * GROUP
                    vector.tensor_tensor(
                        out=m1[:], in0=stage[:, ra:ra + 2048],
                        in1=stage[:, rb:rb + 2048],
                        op=MAX).then_inc(stage_free, 4)
                else:  # f2
                    o = op['out']
                    ob = obuf[:, o['c'] * SLOTS_DEV + o['base']:
                              o['c'] * SLOTS_DEV + o['base'] + o['nslots']]
                    vector.tensor_tensor(out=ob, in0=m1[:, :1024],
                                         in1=m1[:, 1024:],
                                         op=MAX).then_inc(out_sem, 1)
    return nc


def _get_nc():
    global _NC_CACHE
    if _NC_CACHE is None:
        _NC_CACHE = _build_nc()
    return _NC_CACHE


def _run_device(query_feature, feature_bank, trace=False):
    qT = np.ascontiguousarray(query_feature.astype(np.float32).T
                              ).astype(FP8)
    in_maps = []
    for i in range(N_CORES):
        shard = feature_bank[i * N_SHARD:(i + 1) * N_SHARD].astype(np.float32)
        bt = np.zeros((D, QOFF + NCOL), dtype=FP8)
        bt[:, :QOFF] = qT
        bt[:, QOFF:QOFF + N_SHARD] = np.ascontiguousarray(shard.T
                                                          ).astype(FP8)
        in_maps.append({"bankT": bt})
    nc = _get_nc()
    res = run_bass_kernel_spmd(nc, in_maps, list(range(N_CORES)), trace=trace)
    bm_dev = np.stack([res.results[i]["blockmax"].astype(np.float32)
                       for i in range(N_CORES)])  # [8, 256, SLOTS_DEV]
    raw = np.stack([res.results[i]["rawsims"].astype(np.float32)
                    for i in range(N_CORES)])     # [8, 256, N_RAW*GROUP]
    raw_bm = raw.reshape(N_CORES, B, N_RAW * GROUP // BLK, BLK).max(axis=3)
    bm = np.concatenate([bm_dev, raw_bm], axis=2)  # [8, 256, SLOTS_C]
    return bm, res


def _slot_rows():
    """[SLOTS_C, BLK] local col idx per blockmax slot (same both chunks)."""
    steps, dve_ops, outs, _ = _plan()
    rows = np.empty((SLOTS_C, BLK), dtype=np.int64)
    G = GROUP
    for o in outs:
        if o['c'] != 0:
            continue
        b0, ns = o['base'], o['nslots']
        op = o['op']
        if op['op'] == 'direct':
            st = op['st']
            j = np.arange(ns)
            rows[b0:b0 + ns] = (st['g'] * G + BLK * j[:, None]
                                + np.arange(BLK)[None, :])
        else:  # f2 (quad output)
            ga, gb, gc, gd = [x['g'] for x in op['q']['grp']]
            j = np.arange(1024)
            rows[b0:b0 + 1024] = np.stack(
                [ga * G + j, gb * G + j, gc * G + j, gd * G + j], axis=1)
    raw_gs = [st['g'] for st in steps
              if st['c'] == 0 and st['kind'] == 'A' and st['raw']]
    assert len(raw_gs) == N_RAW
    per = GROUP // BLK
    for k, g in enumerate(raw_gs):
        b0 = SLOTS_DEV + k * per
        j = np.arange(per)
        rows[b0:b0 + per] = (g * G + BLK * j[:, None]
                             + np.arange(BLK)[None, :])
    return rows


def _host_topk(bm, query_feature, feature_bank, nsel=512):
    """bm: [8, 256, SLOTS_C] f32 device blockmaxima. Returns top-K indices
    [B, K] into the full bank, matching f32 jax top_k semantics."""
    q = query_feature.astype(np.float32)
    fb = feature_bank.astype(np.float32)
    srows = _slot_rows()
    grow_flat = np.empty((N_CORES * SLOTS_C, BLK), dtype=np.int64)
    for cidx in range(N_CORES):
        g = srows + cidx * N_SHARD
        g[srows >= N_SHARD] = N_TOTAL
        grow_flat[cidx * SLOTS_C:(cidx + 1) * SLOTS_C] = g
    bm_flat = bm.transpose(1, 0, 2).reshape(B, N_CORES * SLOTS_C)
    fb_pad = np.vstack([fb, np.zeros((1, D), np.float32)])

    order = np.argsort(-bm_flat, axis=1)
    sel_sorted = np.take_along_axis(bm_flat, order, axis=1)
    topk_idx = np.empty((B, K), dtype=np.int64)
    pending = np.arange(B)
    nb = nsel
    while len(pending):
        nb = min(nb, bm_flat.shape[1])
        rows = grow_flat[order[pending, :nb]].reshape(len(pending), -1)
        sims = np.einsum("qrd,qd->qr", fb_pad[rows], q[pending],
                         optimize=True)
        sims[rows == N_TOTAL] = -np.inf
        still = []
        for j, b in enumerate(pending):
            o = np.lexsort((rows[j], -sims[j]))[:K]
            tK = sims[j][o[-1]]
            unsel = sel_sorted[b, nb] if nb < bm_flat.shape[1] else -np.inf
            if unsel + MARGIN < tK or nb >= bm_flat.shape[1]:
                topk_idx[b] = rows[j][o]
            else:
                still.append(b)
        pending = np.array(still, dtype=np.int64)
        nb *= 2
    return topk_idx


def _labels_to_output(topk_idx, target_bank):
    tb = np.asarray(target_bank).astype(np.int64)
    out = np.empty((B, NUM_CLASSES), dtype=np.int32)
    allc = np.arange(NUM_CLASSES)
    for b in range(B):
        mask = np.zeros(NUM_CLASSES, dtype=bool)
        mask[tb[topk_idx[b]]] = True
        out[b, :mask.sum()] = allc[mask]
        out[b, mask.sum():] = allc[~mask]
    return out


def kernel(query_feature, feature_bank, target_bank):
    query_feature = np.asarray(query_feature)
    feature_bank = np.asarray(feature_bank)
    target_bank = np.asarray(target_bank)
    bm, _ = _run_device(query_feature, feature_bank)
    topk_idx = _host_topk(bm, query_feature, feature_bank)
    return _labels_to_output(topk_idx, target_bank)


# revision 50
# speedup vs baseline: 1.0266x; 1.0266x over previous
"""Distributed KNN online evaluator kernel for 8 trn2 NeuronCores.

Device side (SPMD over 8 cores, bank sharded over N):
  - bank shard (+queries) resident in SBUF as fp8e4m3, loaded via 7
    chained DMAs (fp8 halves HBM traffic; margin covers the quantization)
  - fp8 matmuls (queries stationary per chunk) -> f32 PSUM,
    1024-col groups on a ring-4 PSUM (fine granularity keeps the
    tensor/ACT/DVE pipeline decoupled; coarser evac units serialize)
  - evacuation split 16 ACT-groups : 8 DVE-groups (+tail) per chunk:
      * D-groups: DVE tensor_reduce blockmax-4 straight from PSUM
      * A-groups: ACT f32->bf16 copy to an 8-slot SBUF stage ring; DVE
        folds 4 staged groups per quad with a 2-level TT max tree
        (2x packed bf16) -> blockmax-4
  - batched DMA out of per-(query, block-of-4) maxima as bf16

Host side:
  - adaptive drill-down: select blocks whose blockmax could contain a
    global top-K sim, recompute those sims exactly in f32, take top-K
  - verified: every unselected block provably below the top-K threshold
    (margin covers fp8/matmul fuzz); expands selection until proven
  - class votes with inf weights degenerate to membership -> output is
    [voted classes asc, unvoted classes asc] per query
"""

import numpy as np
import ml_dtypes

import concourse.bass as bass
import concourse.mybir as mybir
from concourse.bass_utils import run_bass_kernel_spmd

BF16 = ml_dtypes.bfloat16
FP8 = ml_dtypes.float8_e4m3fn

N_CORES = 8
B = 256
D = 128
N_TOTAL = 200000
N_SHARD = N_TOTAL // N_CORES   # 25000
GROUP = 1024
N_FULL = 24
TAIL = 512
NCOL = N_FULL * GROUP + TAIL   # 25088
QOFF = 256
N_STEPS_C = N_FULL + 1
BLK = 4
SLOTS_C = NCOL // BLK          # 6272 total screen slots per chunk
N_RAW = 14                     # A-groups per chunk shipped raw to HBM
N_FOLD = 0                     # no DVE folds: every A-group ships raw
SLOTS_DEV = SLOTS_C - N_RAW * (GROUP // BLK)   # 2688 device blockmax slots
RAW_RING = 8                   # rawbuf ring slots x 1024 bf16
K = 200
NUM_CLASSES = 1000
MARGIN = 3.5                   # fp8 sim err measured |max| 2.7 over 12.8M

PSUM_RING = 4
STAGE_RING = 8                 # stage: 8 slots x 1024 bf16
# 14 A + 10 D per chunk (+tail): ACT is the binding engine at 16 A
# (proven: removing DVE work alone changed nothing), so shift 2 groups
# to DVE direct-reduce and keep DVE in budget by shipping 6 of the 14
# A-groups raw to HBM (host blockmaxes them) instead of folding.
_DSET = {0, 2, 5, 7, 10, 12, 15, 17, 20, 22}
ASSIGN = ['D' if i in _DSET else 'A' for i in range(24)]
# bank DMA parts (cols, queue): alternate the HWDGE (sync) and SWDGE
# (gpsimd) rings so one ring's ~2us completion receipt overlaps the
# other ring's transfer (single-ring parts arrived only every ~3.5us).
DMA_PARTS = [(QOFF + GROUP, 's'), (3 * GROUP, 'g'), (3 * GROUP, 's'),
             (3 * GROUP, 'g'), (3 * GROUP, 's'), (3 * GROUP, 'g'),
             (3 * GROUP, 's'), (3 * GROUP, 'g'), (2 * GROUP + TAIL, 's')]
assert sum(c for c, _ in DMA_PARTS) == QOFF + NCOL
OUT_BATCH = 3                  # DVE output units per out-DMA

_NC_CACHE = None


def _plan():
    steps = []
    for c in range(2):
        na_c = 0
        for g in range(N_STEPS_C):
            kind = 'T' if g == N_FULL else ASSIGN[g]
            st = dict(c=c, g=g, kind=kind, s=len(steps))
            if kind == 'A':
                na_c += 1
                # first N_FOLD A-groups fold on DVE, the rest ship raw
                st['raw'] = na_c > N_FOLD
            steps.append(st)
    na = nf = nr = 0
    for st in steps:
        if st['kind'] == 'A':
            na += 1
            st['na'] = na          # global A order (evacA threshold)
            if st['raw']:
                st['nr'] = nr      # raw copy order (rawbuf slot / DMA)
                nr += 1
            else:
                st['nf'] = nf      # fold copy order (stage slot)
                nf += 1
    # DVE atoms: directs get priority (they recycle PSUM ring slots);
    # quad folds are split into f1/f2 atoms drained one per direct so no
    # single DVE block delays the ring by more than ~1.2us.
    dve_ops = []
    for c in range(2):
        pend = []
        fold_fifo = []
        for st in steps:
            if st['c'] != c:
                continue
            if st['kind'] in ('D', 'T'):
                dve_ops.append(dict(op='direct', st=st))
                if fold_fifo:
                    dve_ops.append(fold_fifo.pop(0))
            elif not st['raw']:
                pend.append(st)
                if len(pend) == 4:
                    q = dict(grp=pend)
                    fold_fifo.append(dict(op='f1', q=q))
                    fold_fifo.append(dict(op='f2', q=q))
                    pend = []
        dve_ops.extend(fold_fifo)
        assert not pend
    outs = []
    base = {0: 0, 1: 0}
    nd = nu = 0
    for op in dve_ops:
        if op['op'] == 'direct':
            st = op['st']
            c = st['c']
            nslots = (TAIL if st['kind'] == 'T' else GROUP) // BLK
            nd += 1
            outs.append(dict(op=op, c=c, base=base[c], nslots=nslots,
                             sem='evacD', cnt=nd))
        elif op['op'] == 'f2':
            c = op['q']['grp'][0]['c']
            nslots = 1024
            nu += 1
            outs.append(dict(op=op, c=c, base=base[c], nslots=nslots,
                             sem='out', cnt=nu))
        else:
            continue
        op['out'] = outs[-1]
        base[c] += outs[-1]['nslots']
    assert base[0] == SLOTS_DEV and base[1] == SLOTS_DEV, base
    # batch consecutive outs (same chunk) into single DMAs
    batches = []
    cur = []
    for o in outs:
        if cur and (len(cur) >= OUT_BATCH or cur[-1]['c'] != o['c']):
            batches.append(cur)
            cur = []
        cur.append(o)
    if cur:
        batches.append(cur)
    return steps, dve_ops, outs, batches


def _part_of(g):
    acc = -QOFF
    for pi, (cols, _) in enumerate(DMA_PARTS):
        acc += cols
        if g * GROUP < acc:
            return pi
    return len(DMA_PARTS) - 1


def _dma_need(p):
    """(n_sync_parts, n_gp_parts) among parts 0..p inclusive."""
    ns = sum(1 for c, q in DMA_PARTS[:p + 1] if q == 's')
    ng = (p + 1) - ns
    return ns, ng


def _build_nc():
    steps, dve_ops, outs, batches = _plan()
    nc = bass.Bass("TRN2", target_bir_lowering=False, debug=False,
                   num_devices=N_CORES)
    bankT = nc.dram_tensor("bankT", [D, QOFF + NCOL], mybir.dt.float8e4,
                           kind="ExternalInput").ap()
    out = nc.dram_tensor("blockmax", [B, SLOTS_DEV], mybir.dt.bfloat16,
                         kind="ExternalOutput").ap()
    raws = nc.dram_tensor("rawsims", [B, N_RAW * GROUP], mybir.dt.bfloat16,
                          kind="ExternalOutput").ap()

    MAX = mybir.AluOpType.max

    with (
        nc.sbuf_tensor([D, QOFF + NCOL], mybir.dt.float8e4) as banks,
        nc.psum_tensor([128, PSUM_RING * GROUP], mybir.dt.float32) as psum,
        nc.sbuf_tensor([128, STAGE_RING * GROUP], mybir.dt.bfloat16) as stage,
        nc.sbuf_tensor([128, 2048], mybir.dt.bfloat16) as m1,
        nc.sbuf_tensor([128, 2 * SLOTS_DEV], mybir.dt.bfloat16) as obuf,
        nc.sbuf_tensor([128, RAW_RING * GROUP], mybir.dt.bfloat16) as rawbuf,
        nc.semaphore() as raw_free,
        nc.semaphore() as dmaS_sem,
        nc.semaphore() as dmaG_sem,
        nc.semaphore() as dmao_sem,
        nc.semaphore() as mm_sem,
        nc.semaphore() as evacA,
        nc.semaphore() as evacD,
        nc.semaphore() as out_sem,
        nc.semaphore() as stage_free,
        nc.Block() as block,
    ):
        @block.gpsimd
        def _(gp):
            lo = 0
            for cols, q in DMA_PARTS:
                if q == 'g':
                    gp.dma_start(banks[:, lo:lo + cols],
                                 bankT[:, lo:lo + cols]).then_inc(dmaG_sem, 16)
                lo += cols
            # raw sims: ship ACT-staged bf16 groups to HBM; host blockmaxes
            for st in steps:
                if st['kind'] != 'A' or not st['raw']:
                    continue
                i, c = st['nr'], st['c']
                gp.wait_ge(evacA, st['na'])
                gp.dma_start(
                    raws[c * 128:(c + 1) * 128,
                         (i % N_RAW) * GROUP:(i % N_RAW) * GROUP + GROUP],
                    rawbuf[:, (i % RAW_RING) * GROUP:
                           (i % RAW_RING) * GROUP + GROUP]
                    ).then_inc(raw_free, 16)

        @block.sync
        def _(sync):
            lo = 0
            for cols, q in DMA_PARTS:
                if q == 's':
                    sync.dma_start(banks[:, lo:lo + cols],
                                   bankT[:, lo:lo + cols]
                                   ).then_inc(dmaS_sem, 16)
                lo += cols
            for batch in batches:
                need_d = max([o['cnt'] for o in batch
                              if o['sem'] == 'evacD'], default=0)
                need_u = max([o['cnt'] for o in batch
                              if o['sem'] == 'out'], default=0)
                if need_d:
                    sync.wait_ge(evacD, need_d)
                if need_u:
                    sync.wait_ge(out_sem, need_u)
                c = batch[0]['c']
                b0 = batch[0]['base']
                ns = sum(o['nslots'] for o in batch)
                sync.dma_start(out[c * 128:(c + 1) * 128, b0:b0 + ns],
                               obuf[:, c * SLOTS_DEV + b0:c * SLOTS_DEV + b0 + ns]
                               ).then_inc(dmao_sem, 16)

        @block.tensor
        def _(tensor):
            nA = nD = 0
            evac_of = {}
            wS = wG = 0
            for st in steps:
                s, c, g, kind = st['s'], st['c'], st['g'], st['kind']
                ns_, ng_ = _dma_need(_part_of(g))
                if ns_ > wS:
                    tensor.wait_ge(dmaS_sem, 16 * ns_)
                    wS = ns_
                if ng_ > wG:
                    tensor.wait_ge(dmaG_sem, 16 * ng_)
                    wG = ng_
                if s >= PSUM_RING:
                    sk, cv = evac_of[s - PSUM_RING]
                    tensor.wait_ge(evacA if sk == 'A' else evacD, cv)
                if kind == 'A':
                    nA += 1
                    evac_of[s] = ('A', nA)
                else:
                    nD += 1
                    evac_of[s] = ('D', nD)
                sl = (s % PSUM_RING) * GROUP
                cols = TAIL if kind == 'T' else GROUP
                nmm = cols // 512
                for m in range(nmm):
                    mm = tensor.matmul(
                        psum[:, sl + m * 512: sl + (m + 1) * 512],
                        lhsT=banks[:, c * 128:(c + 1) * 128],
                        rhs=banks[:, QOFF + g * GROUP + m * 512:
                                  QOFF + g * GROUP + (m + 1) * 512],
                        start=True, stop=True)
                    if m == nmm - 1:
                        mm.then_inc(mm_sem, 1)

        @block.scalar
        def _(scalar):
            for st in steps:
                if st['kind'] != 'A':
                    continue
                s = st['s']
                sl = (s % PSUM_RING) * GROUP
                if st['raw']:
                    i = st['nr']
                    if i >= RAW_RING:
                        scalar.wait_ge(raw_free, 16 * (i - (RAW_RING - 1)))
                    scalar.wait_ge(mm_sem, s + 1)
                    rs = (i % RAW_RING) * GROUP
                    scalar.copy(rawbuf[:, rs:rs + GROUP],
                                psum[:, sl:sl + GROUP]).then_inc(evacA, 1)
                else:
                    n = st['nf']
                    if n >= STAGE_RING:
                        scalar.wait_ge(stage_free, n - (STAGE_RING - 1))
                    scalar.wait_ge(mm_sem, s + 1)
                    ss = (n % STAGE_RING) * GROUP
                    scalar.copy(stage[:, ss:ss + GROUP],
                                psum[:, sl:sl + GROUP]).then_inc(evacA, 1)

        @block.vector
        def _(vector):
            for op in dve_ops:
                if op['op'] == 'direct':
                    o = op['out']
                    ob = obuf[:, o['c'] * SLOTS_DEV + o['base']:
                              o['c'] * SLOTS_DEV + o['base'] + o['nslots']]
                    st = op['st']
                    vector.wait_ge(mm_sem, st['s'] + 1)
                    sl = (st['s'] % PSUM_RING) * GROUP
                    cols = TAIL if st['kind'] == 'T' else GROUP
                    vector.tensor_reduce(
                        out=ob,
                        in_=psum[:, sl:sl + cols].rearrange(
                            "p (b w) -> p b w", w=BLK),
                        axis=mybir.AxisListType.X,
                        op=MAX,
                    ).then_inc(evacD, 1)
                elif op['op'] == 'f1':
                    grp = op['q']['grp']
                    vector.wait_ge(evacA, grp[-1]['na'])
                    ra = (grp[0]['nf'] % STAGE_RING) * GROUP
                    rb = (grp[2]['nf'] % STAGE_RING) * GROUP
                    vector.tensor_tensor(
                        out=m1[:], in0=stage[:, ra:ra + 2048],
                        in1=stage[:, rb:rb + 2048],
                        op=MAX).then_inc(stage_free, 4)
                else:  # f2
                    o = op['out']
                    ob = obuf[:, o['c'] * SLOTS_DEV + o['base']:
                              o['c'] * SLOTS_DEV + o['base'] + o['nslots']]
                    vector.tensor_tensor(out=ob, in0=m1[:, :1024],
                                         in1=m1[:, 1024:],
                                         op=MAX).then_inc(out_sem, 1)
    return nc


def _get_nc():
    global _NC_CACHE
    if _NC_CACHE is None:
        _NC_CACHE = _build_nc()
    return _NC_CACHE


def _run_device(query_feature, feature_bank, trace=False):
    qT = np.ascontiguousarray(query_feature.astype(np.float32).T
                              ).astype(FP8)
    in_maps = []
    for i in range(N_CORES):
        shard = feature_bank[i * N_SHARD:(i + 1) * N_SHARD].astype(np.float32)
        bt = np.zeros((D, QOFF + NCOL), dtype=FP8)
        bt[:, :QOFF] = qT
        bt[:, QOFF:QOFF + N_SHARD] = np.ascontiguousarray(shard.T
                                                          ).astype(FP8)
        in_maps.append({"bankT": bt})
    nc = _get_nc()
    res = run_bass_kernel_spmd(nc, in_maps, list(range(N_CORES)), trace=trace)
    bm_dev = np.stack([res.results[i]["blockmax"].astype(np.float32)
                       for i in range(N_CORES)])  # [8, 256, SLOTS_DEV]
    raw = np.stack([res.results[i]["rawsims"].astype(np.float32)
                    for i in range(N_CORES)])     # [8, 256, N_RAW*GROUP]
    raw_bm = raw.reshape(N_CORES, B, N_RAW * GROUP // BLK, BLK).max(axis=3)
    bm = np.concatenate([bm_dev, raw_bm], axis=2)  # [8, 256, SLOTS_C]
    return bm, res


def _slot_rows():
    """[SLOTS_C, BLK] local col idx per blockmax slot (same both chunks)."""
    steps, dve_ops, outs, _ = _plan()
    rows = np.empty((SLOTS_C, BLK), dtype=np.int64)
    G = GROUP
    for o in outs:
        if o['c'] != 0:
            continue
        b0, ns = o['base'], o['nslots']
        op = o['op']
        if op['op'] == 'direct':
            st = op['st']
            j = np.arange(ns)
            rows[b0:b0 + ns] = (st['g'] * G + BLK * j[:, None]
                                + np.arange(BLK)[None, :])
        else:  # f2 (quad output)
            ga, gb, gc, gd = [x['g'] for x in op['q']['grp']]
            j = np.arange(1024)
            rows[b0:b0 + 1024] = np.stack(
                [ga * G + j, gb * G + j, gc * G + j, gd * G + j], axis=1)
    raw_gs = [st['g'] for st in steps
              if st['c'] == 0 and st['kind'] == 'A' and st['raw']]
    assert len(raw_gs) == N_RAW
    per = GROUP // BLK
    for k, g in enumerate(raw_gs):
        b0 = SLOTS_DEV + k * per
        j = np.arange(per)
        rows[b0:b0 + per] = (g * G + BLK * j[:, None]
                             + np.arange(BLK)[None, :])
    return rows


def _host_topk(bm, query_feature, feature_bank, nsel=512):
    """bm: [8, 256, SLOTS_C] f32 device blockmaxima. Returns top-K indices
    [B, K] into the full bank, matching f32 jax top_k semantics."""
    q = query_feature.astype(np.float32)
    fb = feature_bank.astype(np.float32)
    srows = _slot_rows()
    grow_flat = np.empty((N_CORES * SLOTS_C, BLK), dtype=np.int64)
    for cidx in range(N_CORES):
        g = srows + cidx * N_SHARD
        g[srows >= N_SHARD] = N_TOTAL
        grow_flat[cidx * SLOTS_C:(cidx + 1) * SLOTS_C] = g
    bm_flat = bm.transpose(1, 0, 2).reshape(B, N_CORES * SLOTS_C)
    fb_pad = np.vstack([fb, np.zeros((1, D), np.float32)])

    order = np.argsort(-bm_flat, axis=1)
    sel_sorted = np.take_along_axis(bm_flat, order, axis=1)
    topk_idx = np.empty((B, K), dtype=np.int64)
    pending = np.arange(B)
    nb = nsel
    while len(pending):
        nb = min(nb, bm_flat.shape[1])
        rows = grow_flat[order[pending, :nb]].reshape(len(pending), -1)
        sims = np.einsum("qrd,qd->qr", fb_pad[rows], q[pending],
                         optimize=True)
        sims[rows == N_TOTAL] = -np.inf
        still = []
        for j, b in enumerate(pending):
            o = np.lexsort((rows[j], -sims[j]))[:K]
            tK = sims[j][o[-1]]
            unsel = sel_sorted[b, nb] if nb < bm_flat.shape[1] else -np.inf
            if unsel + MARGIN < tK or nb >= bm_flat.shape[1]:
                topk_idx[b] = rows[j][o]
            else:
                still.append(b)
        pending = np.array(still, dtype=np.int64)
        nb *= 2
    return topk_idx


def _labels_to_output(topk_idx, target_bank):
    tb = np.asarray(target_bank).astype(np.int64)
    out = np.empty((B, NUM_CLASSES), dtype=np.int32)
    allc = np.arange(NUM_CLASSES)
    for b in range(B):
        mask = np.zeros(NUM_CLASSES, dtype=bool)
        mask[tb[topk_idx[b]]] = True
        out[b, :mask.sum()] = allc[mask]
        out[b, mask.sum():] = allc[~mask]
    return out


def kernel(query_feature, feature_bank, target_bank):
    query_feature = np.asarray(query_feature)
    feature_bank = np.asarray(feature_bank)
    target_bank = np.asarray(target_bank)
    bm, _ = _run_device(query_feature, feature_bank)
    topk_idx = _host_topk(bm, query_feature, feature_bank)
    return _labels_to_output(topk_idx, target_bank)
